# revision 53
# baseline (speedup 1.0000x reference)
"""Trainium2 Bass kernel for LinearPerformerAttention (causal linear attention).

Sharding: head-parallel across 8 cores (head c -> core c). Each core computes
its head's causal linear attention over all 2048 tokens via chunked prefix
sums (16 chunks of 128 tokens), then a partial output projection
attn_h @ W_out[h*64:(h+1)*64, :].  The host sums the 8 partial (2048,512)
outputs and adds b_out (tensor-parallel unshard).

Design notes (HW-profiled on trn2; PE is the saturated engine):
  * all-bf16 matmuls (1 cycle/row on PE at any moving-dim size; fp32/f32r
    pay 4x below 256 moving cols), f16 output partials (halves DMA).
  * proj_matrix folded into W_q/W_k on host: qp_pre = x @ (Wq pm), so q/k
    themselves are never formed and the [64,128] projection matmuls vanish.
  * v computed directly token-major from xT k-tiles (no per-chunk transpose).
  * elu1p(t) = min(exp(t), 1 + relu(t)): Exp on ACT, 1+relu on DVE, min on
    DVE (GpSimd cannot run TensorTensor on real HW; ACT/DVE are the only
    PSUM-drain engines).
  * denominator rides the numerator matmul as a 65th row (lhsT = full
    [S|z] / [v|1] tiles) -- saves 2 matmuls+ldweights per chunk; the den row
    returns to a [128,1] column via a 1-row PE transpose so the reciprocal
    runs one-element-per-lane (a [1,128] reciprocal is ~6x slower on HW).
  * PSUM banks (bank-granular slots!): 3 rotate the per-chunk small tiles
    (pt | pa | pv | po65), 2 feature-map psums, 2 outproj psums,
    1 persistent S accumulator. Feature and outproj psums get separate tags
    so phase A of block b+1 never WAR-waits on block b's recursion tails.
  * emission interleaves three streams so the PE never idles: the
    S-recursion of block b-1 sits between the two feature-matmul groups of
    block b, with the x-only-dependent v-matmuls sprinkled between
    recursion steps as pure filler (PE stalls reset its p-state ramp; dense
    PE streams run at a visibly higher clock on HW).
  * DMA: host pre-tiles x ([128, blk, kt, 512]); block 0 loads as 4 k-tile
    DMAs (fast start), blocks 1-3 as one DMA each; weights in 2 DMAs with
    everything feat_q(0)/v need in the first; output staged per 2 chunks,
    last two chunks ship individually (final transfer 128KB). HWDGE
    descriptor generation costs ~625ns per DMA, serialized per queue; x
    block-0 k-tiles 0-1 issue from the SCALAR queue (the 2nd HWDGE ring,
    qActDynamicHW) so their gens parallel sync's wcat1.
  * Dummy-MATMUL warm-up DID pre-ramp the clock (~2.5us) but exposed a
    timing-sensitive sync hazard on real hardware (deterministic 3.5e-2
    corruption in one configuration, an intermittent NaN in another; the
    instruction-level interpreter reproduces neither) -- do not reintroduce
    without extensive hardware revalidation.  LDWEIGHTS bursts do NOT ramp
    the clock and just delay nothing (removed; measured-neutral).

Session-2 findings (HW-measured; baseline 52.0us -> 48.2us final):
  * graded window = "main" slice start -> last sequencer slice end.  The
    NRT preamble (~6us of EVENT_SEMAPHORE + TENSOR_LOAD) is FREE; the
    teardown is NOT: ~3.5us per-semaphore clear storm + ~1us instruction
    refill (Q_XIV) + ~4us final barrier/DMA-completion waits ~= 10us fixed
    after the last drain.  Startup ~= 4us after main (DMA completion-sem
    latency ~1.4-2us is the floor, not data time).
  * NEVER issue DMAs from gpsimd SWDGE in this kernel: its descriptor
    rings live in SBUF partitions 0-31 and the 16 SDMA fetchers slowed
    EVERY SBUF op ~20% (ACT 646->762, DVE min 331->424, MMs 278->392;
    HW exec 52->68us).
  * engine queues are strict in-order: ONE emission order drives all five
    queues, so decouple them op-class-wise.  feat's DVE ops (max/add, min)
    are emitted AFTER the neighboring rec chunks' num/s copies (feat_dve
    split); the whole per-chunk tail (pdc transpose/recip/outproj/drain)
    is deferred one chunk (emit_tail) so no PE op waits at the queue head
    on a just-issued DVE copy.  This was worth ~1.5-2.5us.
  * s-copy BEFORE num-copy on DVE (it gates the whole next rec step; the
    num copy's consumers are tail-deferred anyway): S-upd stall avg
    96->63ns, HW 51.3 -> 49.3us mean.
  * v_chunk split into v_mm + deferred v_copy (ACT): the vv copies are
    emitted after the neighboring rec's tail so the pj-bank-releasing
    drains queue clean on ACT.  HW 49.3 -> 48.2-48.3us (best 48195).
  * dead-work elimination: chunk 15's S-update MM + s-copy (state never
    consumed) and prep(15)'s kp transpose/copy removed from the critical
    tail; chunk 0's num1 MM vs the all-zeros initial S skipped (exact).
    HW 48.2 -> 47.5us.
  * rec lag is 2 chunks (not 4): only two bare rec steps remain at the
    end.  In-slice MM durations inflate 2-4x in chain-stalled phases
    (slices include queue-head wait): N=512 216ns warm vs 537ns stalled.
  * FAILED (sim- or HW-measured worse, do not retry blindly):
    fps+pj sharing one 4-deep psum rotation (feat then WAR-waits on old
    drains, sim +6.6us); superchunk pairing with A_ab correction (HW
    54.4us: +8 MM/+8 copies outweigh halved chain round-trips); quad
    output staging (sim +0.7us); wcat1 split (delays sync queue, sim
    +0.7us); vv copies on DVE for tail chunks (HW 52.2); per-chunk output
    DMAs from gpsimd (see SWDGE above).
  * run-to-run HW variance is +/-2us (It4 config: 49.6/51.7/50.9) --
    single runs cannot resolve sub-1us scheduling changes; sim
    (SIM_ONLY=1, TimelineSim) tracks dependency-structure deltas well but
    models neither HAM clock state nor in-slice stalls.
"""

import threading
from contextlib import ExitStack

import numpy as np
import ml_dtypes

import concourse.bass as bass
import concourse.mybir as mybir
import concourse.tile as tile
from concourse import bacc
from concourse.bass_utils import run_bass_kernel_spmd

DIM, HEADS, FEAT = 512, 8, 128
HD = DIM // HEADS          # 64
N = 2048
C = 128                    # chunk (tokens)
NCH = N // C               # 16
NBLK = 4                   # token blocks of 512 for phase A
KT = 4                     # k-tiles of 128 over DIM

F32 = mybir.dt.float32
BF16 = mybir.dt.bfloat16
F16 = mybir.dt.float16
AF = mybir.ActivationFunctionType
ALU = mybir.AluOpType

NP_BF16 = ml_dtypes.bfloat16

# wcat column layout: [wqp(4*128) | wv(4*64) | wkp(4*128) | mask(128) | id(128)]
# first DMA covers wqp+wv (everything feat_q(0) and the v-matmuls need)
WQP0 = 0
WV0 = KT * FEAT                 # 512
WKP0 = WV0 + KT * HD            # 768
WCAT1 = WKP0                    # first-DMA column count
MASK0 = WKP0 + KT * FEAT        # 1280
ID0 = MASK0 + 128               # 1408
WCAT_COLS = ID0 + 128           # 1536



def build_nc():
    nc = bacc.Bacc()

    xT_d = nc.declare_dram_parameter("xt", [128, NBLK, KT, 512], BF16,
                                     isOutput=False)
    wcat_d = nc.declare_dram_parameter("wcat", [128, WCAT_COLS], BF16,
                                       isOutput=False)
    wo_d = nc.declare_dram_parameter("w_out_h", [HD, DIM], BF16, isOutput=False)
    # out[p, c, col] = full_out[token = c*128 + p, col]
    out_d = nc.declare_dram_parameter("out_part", [128, NCH, DIM], F16,
                                      isOutput=True)

    with ExitStack() as ctx:
        tc = ctx.enter_context(tile.TileContext(nc))
        const = ctx.enter_context(tc.tile_pool(name="const", bufs=1))
        fpool = ctx.enter_context(tc.tile_pool(name="feat", bufs=3))
        spool = ctx.enter_context(tc.tile_pool(name="spool", bufs=3))
        ampool = ctx.enter_context(tc.tile_pool(name="am", bufs=8))
        numpool = ctx.enter_context(tc.tile_pool(name="num", bufs=3))
        opool = ctx.enter_context(tc.tile_pool(name="osb", bufs=3))
        dpool = ctx.enter_context(tc.tile_pool(name="dinv", bufs=3))
        # PSUM banks: psml(3) + pbig(2 fps + 2 pj tags) + psp(1) = 8
        psml = ctx.enter_context(tc.tile_pool(name="psml", bufs=3, space="PSUM"))
        pbig = ctx.enter_context(tc.tile_pool(name="pbig", bufs=2, space="PSUM"))
        psp = ctx.enter_context(tc.tile_pool(name="psp", bufs=1, space="PSUM"))

        ps_s = psp.tile([FEAT, HD + 1], F32)   # persistent S accumulator

        # ---- constants; DMA order = first-use order so feat(0) starts early:
        # wqp, x-block0 k-tiles, wkp+wv+mask+id, x-blocks 1-3, wo ----
        wcat = const.tile([128, WCAT_COLS], BF16)
        wo_sb = const.tile([HD, DIM], BF16)
        nc.sync.dma_start(wcat[:, 0:WCAT1], wcat_d[:, 0:WCAT1])

        def wqp(kk):
            return wcat[:, WQP0 + kk * FEAT: WQP0 + (kk + 1) * FEAT]

        def wkp(kk):
            return wcat[:, WKP0 + kk * FEAT: WKP0 + (kk + 1) * FEAT]

        def wv(kk):
            return wcat[:, WV0 + kk * HD: WV0 + (kk + 1) * HD]

        mask_sb = wcat[:, MASK0:MASK0 + 128]
        id_sb = wcat[:, ID0:ID0 + 128]

        s0_sb = const.tile([FEAT, HD + 1], BF16)
        nc.vector.memset(s0_sb[:], 0.0)
        id_one = id_sb[HD:HD + 1, HD:HD + 1]   # [[1]] at base partition 64

        # ---- persistent intermediates ----
        xsb = const.tile([128, NBLK, KT, 512], BF16)   # all of x^T, pre-tiled
        qpT = const.tile([FEAT, N], BF16)              # elu1p(x @ (Wq pm))^T
        kpT = const.tile([FEAT, N], BF16)
        kp_tm = const.tile([128, N], BF16)             # token-major kp
        vv = const.tile([128, NCH, HD + 1], BF16)      # [v | 1] per chunk
        nc.vector.memset(vv[:, :, HD:HD + 1], 1.0)     # ones column, all chunks

        # x DMAs up front; block 0 split by k-tile so featmuls start early.
        # k-tiles 0-1 go through the scalar queue (2nd HWDGE ring) so their
        # ~0.7us descriptor gens run in parallel with sync's wcat1.
        # (NEVER gpsimd SWDGE here: its SBUF descriptor rings + 16 SDMA
        # fetchers on partitions 0-31 slow every SBUF op ~20%, HW-measured.)
        nc.scalar.dma_start(xsb[:, 0, 0, :], xT_d[:, 0, 0, :])
        nc.scalar.dma_start(xsb[:, 0, 1, :], xT_d[:, 0, 1, :])
        nc.sync.dma_start(xsb[:, 0, 2, :], xT_d[:, 0, 2, :])
        nc.sync.dma_start(xsb[:, 0, 3, :], xT_d[:, 0, 3, :])
        nc.sync.dma_start(wcat[:, WCAT1:], wcat_d[:, WCAT1:])
        for blk in range(1, NBLK):
            nc.sync.dma_start(xsb[:, blk, :, :], xT_d[:, blk, :, :])
        nc.sync.dma_start(wo_sb[:], wo_d[:])

        # ---- emission helpers ----
        # feat is split into the MM+EXP part and the DVE part so the DVE
        # ops can be emitted AFTER the S-chain's critical copies: engine
        # queues are in-order, and a rec s-copy queued behind a 600ns
        # maxadd stalls the next S-update (measured as inflated MM slices)
        feat_dve_pending = {}

        def feat_mm(blk, which):
            sl = slice(blk * 512, (blk + 1) * 512)
            wfn, dstT = ((wqp, qpT), (wkp, kpT))[which]
            ps = pbig.tile([FEAT, 512], F32, tag="fps", name=f"fps{blk}_{which}")
            for kk in range(KT):
                nc.tensor.matmul(ps[:], wfn(kk), xsb[:, blk, kk, :],
                                 start=(kk == 0), stop=(kk == KT - 1))
            e = fpool.tile([FEAT, 512], BF16, tag="e", name=f"e{blk}_{which}")
            nc.scalar.activation(e[:], ps[:], AF.Exp)
            feat_dve_pending[(blk, which)] = (ps, e, dstT, sl)

        def feat_dve(blk, which):
            ps, e, dstT, sl = feat_dve_pending.pop((blk, which))
            r = fpool.tile([FEAT, 512], BF16, tag="r", name=f"r{blk}_{which}")
            nc.vector.tensor_scalar(r[:], ps[:], 0.0, 1.0, ALU.max, ALU.add)
            nc.vector.tensor_tensor(dstT[:, sl], e[:], r[:], ALU.min)

        am_tiles = {}

        pv_tiles = {}

        def v_mm(i):
            """Token-major v for chunk i, straight from xT k-tiles. Depends
            only on the x DMA -- used as PE filler between recursion steps."""
            blk, sub = i // 4, i % 4
            cs = slice(sub * C, (sub + 1) * C)
            pv = psml.tile([128, HD], F32, tag="sml", name=f"pv{i}")
            for kk in range(KT):
                nc.tensor.matmul(pv[:], xsb[:, blk, kk, cs], wv(kk),
                                 start=(kk == 0), stop=(kk == KT - 1))
            pv_tiles[i] = pv

        def v_copy(i):
            # ACT copy (not DVE), emitted AFTER the neighboring rec's tail
            # so the pj-bank-releasing drains queue clean on ACT
            nc.scalar.activation(vv[:, i, 0:HD], pv_tiles.pop(i)[:], AF.Copy)

        def prep_chunk(i):
            """S-independent per-chunk work: kp transpose, masked A^T.
            Chunk 15's kp_tm only feeds the (dead) final S-update -- skip."""
            ci = slice(i * C, (i + 1) * C)
            if i < NCH - 1:
                pt = psml.tile([128, 128], BF16, tag="sml", name=f"pt{i}")
                nc.tensor.transpose(pt[:], kpT[:, ci], id_sb)
                nc.vector.tensor_copy(kp_tm[:, ci], pt[:])
            pa = psml.tile([128, 128], F32, tag="sml", name=f"pa{i}")
            nc.tensor.matmul(pa[:], kpT[:, ci], qpT[:, ci], start=True, stop=True)
            am = ampool.tile([128, 128], BF16, name=f"am{i}")
            nc.vector.tensor_tensor(am[:], pa[:], mask_sb, ALU.mult)
            am_tiles[i] = am

        osb2 = [None]

        def emit_tail(num, i):
            # whole tail of chunk i runs a full chunk after its num copy:
            # pdc/outproj never sit at the PE queue head waiting on DVE
            pdc = psml.tile([128, 1], BF16, tag="sml", name=f"pdc{i}")
            nc.tensor.transpose(pdc[:], num[HD:HD + 1, :], id_one)
            dinv = dpool.tile([128, 1], F32, name=f"dinv{i}")
            nc.vector.reciprocal(dinv[:], pdc[:])
            pj = pbig.tile([128, DIM], F32, tag="pj", name=f"pj{i}")
            nc.tensor.matmul(pj[:], num[0:HD, :], wo_sb[:], start=True, stop=True)
            if i >= NCH - 2:
                # last two chunks ship individually: chunk 14's DMA starts a
                # drain earlier and the final transfer is only 128KB
                osb = opool.tile([128, 1, DIM], F16, name=f"osb1_{i}")
                nc.scalar.activation(osb[:, 0, :], pj[:], AF.Copy,
                                     scale=dinv[:])
                nc.sync.dma_start(out_d[:, i:i + 1, :], osb[:])
                return
            if i % 2 == 0:
                osb2[0] = opool.tile([128, 2, DIM], F16, name=f"osb2_{i}")
            osb = osb2[0]
            nc.scalar.activation(osb[:, i % 2, :], pj[:], AF.Copy,
                                 scale=dinv[:])
            if i % 2 == 1:
                nc.sync.dma_start(out_d[:, i - 1:i + 1, :], osb[:])

        state = {"s_prev": s0_sb, "pending": None}

        def rec_chunk(i):
            """S-chain per-chunk work + pipelined tail of chunk i-1."""
            blk, sub = i // 4, i % 4
            ci = slice(i * C, (i + 1) * C)
            cs = slice(sub * C, (sub + 1) * C)
            s_prev = state["s_prev"]
            am = am_tiles.pop(i)
            # numerator psum tile; row 64 = denominator^T
            pon = psml.tile([HD + 1, 128], F32, tag="sml", name=f"pk{i}")
            po65 = pon[:]
            # S' += kp_tm^T @ [v|1]  (PSUM accumulation across chunks).
            # The final chunk's S-update/snapshot have no consumer -- skip
            # both (they sat on the critical tail's PE and DVE queues).
            if i < NCH - 1:
                nc.tensor.matmul(ps_s[:], kp_tm[:, ci], vv[:, i, :],
                                 start=(i == 0), stop=(i == NCH - 2),
                                 skip_group_check=True)
            # [num^T; den^T] [65, ti] = [S|z]^T qpc + [v|1]^T am.
            # Chunk 0: s_prev is all-zeros -- its matmul contributes exactly
            # nothing (including the den row: z0 = 0), so skip it.
            if i > 0:
                nc.tensor.matmul(po65, s_prev[:], qpT[:, ci],
                                 start=True, stop=False, skip_group_check=True)
            nc.tensor.matmul(po65, vv[:, i, :], am[:],
                             start=(i == 0), stop=True, skip_group_check=True)
            # S snapshot FIRST on DVE: it gates the whole next rec step
            # (num1 lhsT RAW + S-update WAR), while the num copy's only
            # consumers (pdc/outproj) are deferred a full chunk in the tail
            if i < NCH - 1:
                s_new = spool.tile([FEAT, HD + 1], BF16, name=f"s{i}")
                nc.vector.tensor_copy(s_new[:], ps_s[:])
            else:
                s_new = s_prev
            num = numpool.tile([HD + 1, 128], BF16, name=f"num{i}")
            nc.vector.tensor_copy(num[:], po65)
            # previous chunk's tail here: covers num-copy latency
            if state["pending"] is not None:
                emit_tail(*state["pending"])
            state["pending"] = (num, i)
            state["s_prev"] = s_new

        # ---- emission schedule (lag-2): the S-recursion of chunk i runs
        # two chunks behind its prep, so only TWO bare rec steps remain at
        # the end (the bare-rec tail is where the PE goes gappy and the HAM
        # clock-gate re-throttles to 1.2GHz -- measured 537ns N=512 MMs).
        # v/feat/prep matmuls are interleaved as filler between rec steps
        # so the PE stays dense while DVE/ACT service the S-chain. ----
        feat_mm(0, 0)
        v_mm(0)
        v_mm(1)
        feat_dve(0, 0)
        v_copy(0)
        v_copy(1)
        v_mm(2)
        v_mm(3)
        feat_mm(0, 1)
        feat_dve(0, 1)
        v_copy(2)
        v_copy(3)
        prep_chunk(0)
        prep_chunk(1)
        rec_chunk(0)
        prep_chunk(2)
        rec_chunk(1)
        prep_chunk(3)
        for blk in range(1, NBLK):
            feat_mm(blk, 0)
            rec_chunk(4 * blk - 2)
            v_mm(4 * blk + 0)
            rec_chunk(4 * blk - 1)
            v_copy(4 * blk + 0)
            v_mm(4 * blk + 1)
            feat_dve(blk, 0)
            feat_mm(blk, 1)
            v_copy(4 * blk + 1)
            feat_dve(blk, 1)
            prep_chunk(4 * blk + 0)
            prep_chunk(4 * blk + 1)
            rec_chunk(4 * blk + 0)
            v_mm(4 * blk + 2)
            rec_chunk(4 * blk + 1)
            v_copy(4 * blk + 2)
            v_mm(4 * blk + 3)
            prep_chunk(4 * blk + 2)
            prep_chunk(4 * blk + 3)
            v_copy(4 * blk + 3)
        rec_chunk(14)
        rec_chunk(15)
        emit_tail(*state["pending"])

    nc.compile()
    return nc


_cache = threading.Lock()
_nc = None


def _get_nc():
    global _nc
    with _cache:
        if _nc is None:
            _nc = build_nc()
    return _nc


def _in_maps(x, proj_matrix, W_qkv, W_out):
    # x^T pre-tiled: [512, 2048] -> [kt, 128, blk, 512] -> [128, blk, kt, 512]
    xT = np.ascontiguousarray(
        x[0].T.reshape(KT, 128, NBLK, 512).transpose(1, 2, 0, 3)
    ).astype(NP_BF16)
    mask = (np.arange(128)[:, None] <= np.arange(128)[None, :]).astype(np.float32)
    ident = np.eye(128, dtype=np.float32)

    def ktile(cols):
        # (512, m) -> (128, 4*m) k-tile layout, kt-major columns
        m = cols.shape[1]
        return cols.reshape(KT, 128, m).transpose(1, 0, 2).reshape(128, KT * m)

    maps = []
    for c in range(HEADS):
        pm = proj_matrix[c]                                 # (64, 128)
        wq = W_qkv[:, c * HD:(c + 1) * HD]                  # (512, 64)
        wk = W_qkv[:, DIM + c * HD: DIM + (c + 1) * HD]
        wv_ = W_qkv[:, 2 * DIM + c * HD: 2 * DIM + (c + 1) * HD]
        wcat = np.concatenate(
            [ktile(wq @ pm), ktile(wv_), ktile(wk @ pm), mask, ident],
            axis=1).astype(NP_BF16)
        maps.append({
            "xt": xT,
            "wcat": np.ascontiguousarray(wcat),
            "w_out_h": np.ascontiguousarray(
                W_out[c * HD:(c + 1) * HD, :]).astype(NP_BF16),
        })
    return maps


def kernel(x, proj_matrix, W_qkv, W_out, b_out, _trace=False):
    x = np.asarray(x, dtype=np.float32)
    proj_matrix = np.asarray(proj_matrix, dtype=np.float32)
    W_qkv = np.asarray(W_qkv, dtype=np.float32)
    W_out = np.asarray(W_out, dtype=np.float32)
    b_out = np.asarray(b_out, dtype=np.float32)

    nc = _get_nc()
    maps = _in_maps(x, proj_matrix, W_qkv, W_out)
    res = run_bass_kernel_spmd(nc, maps, core_ids=list(range(HEADS)), trace=_trace)
    out = np.zeros((N, DIM), dtype=np.float32)
    for r in res.results:
        part = np.asarray(r["out_part"], dtype=np.float32)   # [128, 16, 512]
        out += part.transpose(1, 0, 2).reshape(N, DIM)
    out += b_out
    if _trace:
        return out.reshape(1, N, DIM), res
    return out.reshape(1, N, DIM)



# revision 55
# speedup vs baseline: 1.0106x; 1.0106x over previous
"""Trainium2 Bass kernel for LinearPerformerAttention (causal linear attention).

Sharding: head-parallel across 8 cores (head c -> core c). Each core computes
its head's causal linear attention over all 2048 tokens via chunked prefix
sums (16 chunks of 128 tokens), then a partial output projection
attn_h @ W_out[h*64:(h+1)*64, :].  The host sums the 8 partial (2048,512)
outputs and adds b_out (tensor-parallel unshard).

Design notes (HW-profiled on trn2; PE is the saturated engine):
  * all-bf16 matmuls (1 cycle/row on PE at any moving-dim size; fp32/f32r
    pay 4x below 256 moving cols), f16 output partials (halves DMA).
  * proj_matrix folded into W_q/W_k on host: qp_pre = x @ (Wq pm), so q/k
    themselves are never formed and the [64,128] projection matmuls vanish.
  * v computed directly token-major from xT k-tiles (no per-chunk transpose).
  * elu1p(t) = min(exp(t), 1 + relu(t)): Exp on ACT, 1+relu on DVE, min on
    DVE (GpSimd cannot run TensorTensor on real HW; ACT/DVE are the only
    PSUM-drain engines).
  * denominator rides the numerator matmul as a 65th row (lhsT = full
    [S|z] / [v|1] tiles) -- saves 2 matmuls+ldweights per chunk; the den row
    returns to a [128,1] column via a 1-row PE transpose so the reciprocal
    runs one-element-per-lane (a [1,128] reciprocal is ~6x slower on HW).
  * PSUM banks (bank-granular slots!): 3 rotate the per-chunk small tiles
    (pt | pa | pv | po65), 2 feature-map psums, 2 outproj psums,
    1 persistent S accumulator. Feature and outproj psums get separate tags
    so phase A of block b+1 never WAR-waits on block b's recursion tails.
  * emission interleaves three streams so the PE never idles: the
    S-recursion of block b-1 sits between the two feature-matmul groups of
    block b, with the x-only-dependent v-matmuls sprinkled between
    recursion steps as pure filler (PE stalls reset its p-state ramp; dense
    PE streams run at a visibly higher clock on HW).
  * DMA: host pre-tiles x ([128, blk, kt, 512]); block 0 loads as 4 k-tile
    DMAs (fast start), blocks 1-3 as one DMA each; weights in 2 DMAs with
    everything feat_q(0)/v need in the first; output staged per 2 chunks,
    last two chunks ship individually (final transfer 128KB). HWDGE
    descriptor generation costs ~625ns per DMA, serialized per queue; x
    block-0 k-tiles 0-1 issue from the SCALAR queue (the 2nd HWDGE ring,
    qActDynamicHW) so their gens parallel sync's wcat1.
  * Dummy-MATMUL warm-up DID pre-ramp the clock (~2.5us) but exposed a
    timing-sensitive sync hazard on real hardware (deterministic 3.5e-2
    corruption in one configuration, an intermittent NaN in another; the
    instruction-level interpreter reproduces neither) -- do not reintroduce
    without extensive hardware revalidation.  LDWEIGHTS bursts do NOT ramp
    the clock and just delay nothing (removed; measured-neutral).

Session-2 findings (HW-measured; baseline 52.0us -> 48.2us final):
  * graded window = "main" slice start -> last sequencer slice end.  The
    NRT preamble (~6us of EVENT_SEMAPHORE + TENSOR_LOAD) is FREE; the
    teardown is NOT: ~3.5us per-semaphore clear storm + ~1us instruction
    refill (Q_XIV) + ~4us final barrier/DMA-completion waits ~= 10us fixed
    after the last drain.  Startup ~= 4us after main (DMA completion-sem
    latency ~1.4-2us is the floor, not data time).
  * NEVER issue DMAs from gpsimd SWDGE in this kernel: its descriptor
    rings live in SBUF partitions 0-31 and the 16 SDMA fetchers slowed
    EVERY SBUF op ~20% (ACT 646->762, DVE min 331->424, MMs 278->392;
    HW exec 52->68us).
  * engine queues are strict in-order: ONE emission order drives all five
    queues, so decouple them op-class-wise.  feat's DVE ops (max/add, min)
    are emitted AFTER the neighboring rec chunks' num/s copies (feat_dve
    split); the whole per-chunk tail (pdc transpose/recip/outproj/drain)
    is deferred one chunk (emit_tail) so no PE op waits at the queue head
    on a just-issued DVE copy.  This was worth ~1.5-2.5us.
  * s-copy BEFORE num-copy on DVE (it gates the whole next rec step; the
    num copy's consumers are tail-deferred anyway): S-upd stall avg
    96->63ns, HW 51.3 -> 49.3us mean.
  * v_chunk split into v_mm + deferred v_copy (ACT): the vv copies are
    emitted after the neighboring rec's tail so the pj-bank-releasing
    drains queue clean on ACT.  HW 49.3 -> 48.2-48.3us (best 48195).
  * dead-work elimination: chunk 15's S-update MM + s-copy (state never
    consumed) and prep(15)'s kp transpose/copy removed from the critical
    tail; chunk 0's num1 MM vs the all-zeros initial S skipped (exact).
    HW 48.2 -> 47.5us.
  * rec lag is 2 chunks (not 4): only two bare rec steps remain at the
    end.  In-slice MM durations inflate 2-4x in chain-stalled phases
    (slices include queue-head wait): N=512 216ns warm vs 537ns stalled.
  * FAILED (sim- or HW-measured worse, do not retry blindly):
    fps+pj sharing one 4-deep psum rotation (feat then WAR-waits on old
    drains, sim +6.6us); superchunk pairing with A_ab correction (HW
    54.4us: +8 MM/+8 copies outweigh halved chain round-trips); quad
    output staging (sim +0.7us); wcat1 split (delays sync queue, sim
    +0.7us); vv copies on DVE for tail chunks (HW 52.2); per-chunk output
    DMAs from gpsimd (see SWDGE above).
  * run-to-run HW variance is +/-2us (It4 config: 49.6/51.7/50.9) --
    single runs cannot resolve sub-1us scheduling changes; sim
    (SIM_ONLY=1, TimelineSim) tracks dependency-structure deltas well but
    models neither HAM clock state nor in-slice stalls.
"""

import threading
from contextlib import ExitStack

import numpy as np
import ml_dtypes

import concourse.bass as bass
import concourse.mybir as mybir
import concourse.tile as tile
from concourse import bacc
from concourse.bass_utils import run_bass_kernel_spmd

DIM, HEADS, FEAT = 512, 8, 128
HD = DIM // HEADS          # 64
N = 2048
C = 128                    # chunk (tokens)
NCH = N // C               # 16
NBLK = 4                   # token blocks of 512 for phase A
KT = 4                     # k-tiles of 128 over DIM

F32 = mybir.dt.float32
BF16 = mybir.dt.bfloat16
F16 = mybir.dt.float16
AF = mybir.ActivationFunctionType
ALU = mybir.AluOpType

NP_BF16 = ml_dtypes.bfloat16

# wcat column layout: [wqp(4*128) | wv(4*64) | wkp(4*128) | mask(128) | id(128)]
# first DMA covers wqp+wv (everything feat_q(0) and the v-matmuls need)
WQP0 = 0
WV0 = KT * FEAT                 # 512
WKP0 = WV0 + KT * HD            # 768
WCAT1 = WKP0                    # first-DMA column count
MASK0 = WKP0 + KT * FEAT        # 1280
ID0 = MASK0 + 128               # 1408
WCAT_COLS = ID0 + 128           # 1536



def build_nc():
    nc = bacc.Bacc()

    xT_d = nc.declare_dram_parameter("xt", [128, NBLK, KT, 512], BF16,
                                     isOutput=False)
    wcat_d = nc.declare_dram_parameter("wcat", [128, WCAT_COLS], BF16,
                                       isOutput=False)
    wo_d = nc.declare_dram_parameter("w_out_h", [HD, DIM], BF16, isOutput=False)
    # out[p, c, col] = full_out[token = c*128 + p, col]
    out_d = nc.declare_dram_parameter("out_part", [128, NCH, DIM], F16,
                                      isOutput=True)

    with ExitStack() as ctx:
        tc = ctx.enter_context(tile.TileContext(nc))
        const = ctx.enter_context(tc.tile_pool(name="const", bufs=1))
        fpool = ctx.enter_context(tc.tile_pool(name="feat", bufs=3))
        spool = ctx.enter_context(tc.tile_pool(name="spool", bufs=3))
        ampool = ctx.enter_context(tc.tile_pool(name="am", bufs=8))
        numpool = ctx.enter_context(tc.tile_pool(name="num", bufs=3))
        opool = ctx.enter_context(tc.tile_pool(name="osb", bufs=3))
        dpool = ctx.enter_context(tc.tile_pool(name="dinv", bufs=3))
        # PSUM banks: psml(3) + pbig(2 fps + 2 pj tags) + psp(1) = 8
        psml = ctx.enter_context(tc.tile_pool(name="psml", bufs=3, space="PSUM"))
        pbig = ctx.enter_context(tc.tile_pool(name="pbig", bufs=2, space="PSUM"))
        psp = ctx.enter_context(tc.tile_pool(name="psp", bufs=1, space="PSUM"))

        ps_s = psp.tile([FEAT, HD + 1], F32)   # persistent S accumulator

        # ---- constants; DMA order = first-use order so feat(0) starts early:
        # wqp, x-block0 k-tiles, wkp+wv+mask+id, x-blocks 1-3, wo ----
        wcat = const.tile([128, WCAT_COLS], BF16)
        wo_sb = const.tile([HD, DIM], BF16)
        nc.sync.dma_start(wcat[:, 0:WCAT1], wcat_d[:, 0:WCAT1])

        def wqp(kk):
            return wcat[:, WQP0 + kk * FEAT: WQP0 + (kk + 1) * FEAT]

        def wkp(kk):
            return wcat[:, WKP0 + kk * FEAT: WKP0 + (kk + 1) * FEAT]

        def wv(kk):
            return wcat[:, WV0 + kk * HD: WV0 + (kk + 1) * HD]

        mask_sb = wcat[:, MASK0:MASK0 + 128]
        id_sb = wcat[:, ID0:ID0 + 128]

        s0_sb = const.tile([FEAT, HD + 1], BF16)
        nc.vector.memset(s0_sb[:], 0.0)
        id_one = id_sb[HD:HD + 1, HD:HD + 1]   # [[1]] at base partition 64

        # ---- persistent intermediates ----
        xsb = const.tile([128, NBLK, KT, 512], BF16)   # all of x^T, pre-tiled
        qpT = const.tile([FEAT, N], BF16)              # elu1p(x @ (Wq pm))^T
        kpT = const.tile([FEAT, N], BF16)
        kp_tm = const.tile([128, N], BF16)             # token-major kp
        vv = const.tile([128, NCH, HD + 1], BF16)      # [v | 1] per chunk
        nc.vector.memset(vv[:, :, HD:HD + 1], 1.0)     # ones column, all chunks

        # x DMAs up front; block 0 split by k-tile so featmuls start early.
        # k-tiles 0-1 go through the scalar queue (2nd HWDGE ring) so their
        # ~0.7us descriptor gens run in parallel with sync's wcat1.
        # (NEVER gpsimd SWDGE here: its SBUF descriptor rings + 16 SDMA
        # fetchers on partitions 0-31 slow every SBUF op ~20%, HW-measured.)
        nc.scalar.dma_start(xsb[:, 0, 0, :], xT_d[:, 0, 0, :])
        nc.scalar.dma_start(xsb[:, 0, 1, :], xT_d[:, 0, 1, :])
        nc.sync.dma_start(xsb[:, 0, 2, :], xT_d[:, 0, 2, :])
        nc.sync.dma_start(xsb[:, 0, 3, :], xT_d[:, 0, 3, :])
        nc.sync.dma_start(wcat[:, WCAT1:], wcat_d[:, WCAT1:])
        for blk in range(1, NBLK):
            nc.sync.dma_start(xsb[:, blk, :, :], xT_d[:, blk, :, :])
        nc.sync.dma_start(wo_sb[:], wo_d[:])

        # ---- emission helpers ----
        # feat is split into the MM+EXP part and the DVE part so the DVE
        # ops can be emitted AFTER the S-chain's critical copies: engine
        # queues are in-order, and a rec s-copy queued behind a 600ns
        # maxadd stalls the next S-update (measured as inflated MM slices)
        feat_dve_pending = {}

        def feat_mm(blk, which):
            sl = slice(blk * 512, (blk + 1) * 512)
            wfn, dstT = ((wqp, qpT), (wkp, kpT))[which]
            ps = pbig.tile([FEAT, 512], F32, tag="fps", name=f"fps{blk}_{which}")
            for kk in range(KT):
                nc.tensor.matmul(ps[:], wfn(kk), xsb[:, blk, kk, :],
                                 start=(kk == 0), stop=(kk == KT - 1))
            e = fpool.tile([FEAT, 512], BF16, tag="e", name=f"e{blk}_{which}")
            nc.scalar.activation(e[:], ps[:], AF.Exp)
            feat_dve_pending[(blk, which)] = (ps, e, dstT, sl)

        def feat_dve(blk, which):
            ps, e, dstT, sl = feat_dve_pending.pop((blk, which))
            r = fpool.tile([FEAT, 512], BF16, tag="r", name=f"r{blk}_{which}")
            nc.vector.tensor_scalar(r[:], ps[:], 0.0, 1.0, ALU.max, ALU.add)
            nc.vector.tensor_tensor(dstT[:, sl], e[:], r[:], ALU.min)

        am_tiles = {}

        pv_tiles = {}

        def v_mm(i):
            """Token-major v for chunk i, straight from xT k-tiles. Depends
            only on the x DMA -- used as PE filler between recursion steps."""
            blk, sub = i // 4, i % 4
            cs = slice(sub * C, (sub + 1) * C)
            pv = psml.tile([128, HD], F32, tag="sml", name=f"pv{i}")
            for kk in range(KT):
                nc.tensor.matmul(pv[:], xsb[:, blk, kk, cs], wv(kk),
                                 start=(kk == 0), stop=(kk == KT - 1))
            pv_tiles[i] = pv

        def v_copy(i):
            # ACT copy (not DVE), emitted AFTER the neighboring rec's tail
            # so the pj-bank-releasing drains queue clean on ACT
            nc.scalar.activation(vv[:, i, 0:HD], pv_tiles.pop(i)[:], AF.Copy)

        def prep_chunk(i):
            """S-independent per-chunk work: kp transpose, masked A^T.
            Chunk 15's kp_tm only feeds the (dead) final S-update -- skip."""
            ci = slice(i * C, (i + 1) * C)
            if i < NCH - 1:
                pt = psml.tile([128, 128], BF16, tag="sml", name=f"pt{i}")
                nc.tensor.transpose(pt[:], kpT[:, ci], id_sb)
                nc.vector.tensor_copy(kp_tm[:, ci], pt[:])
            pa = psml.tile([128, 128], F32, tag="sml", name=f"pa{i}")
            nc.tensor.matmul(pa[:], kpT[:, ci], qpT[:, ci], start=True, stop=True)
            am = ampool.tile([128, 128], BF16, name=f"am{i}")
            nc.vector.tensor_tensor(am[:], pa[:], mask_sb, ALU.mult)
            am_tiles[i] = am

        osb2 = [None]

        def emit_tail(num, i):
            # whole tail of chunk i runs a full chunk after its num copy:
            # pdc/outproj never sit at the PE queue head waiting on DVE
            pdc = psml.tile([128, 1], BF16, tag="sml", name=f"pdc{i}")
            nc.tensor.transpose(pdc[:], num[HD:HD + 1, :], id_one)
            dinv = dpool.tile([128, 1], F32, name=f"dinv{i}")
            nc.vector.reciprocal(dinv[:], pdc[:])
            pj = pbig.tile([128, DIM], F32, tag="pj", name=f"pj{i}")
            nc.tensor.matmul(pj[:], num[0:HD, :], wo_sb[:], start=True, stop=True)
            if i >= NCH - 2:
                # last two chunks ship individually: chunk 14's DMA starts a
                # drain earlier and the final transfer is only 128KB
                osb = opool.tile([128, 1, DIM], F16, name=f"osb1_{i}")
                nc.scalar.activation(osb[:, 0, :], pj[:], AF.Copy,
                                     scale=dinv[:])
                nc.sync.dma_start(out_d[:, i:i + 1, :], osb[:])
                return
            if i % 2 == 0:
                osb2[0] = opool.tile([128, 2, DIM], F16, name=f"osb2_{i}")
            osb = osb2[0]
            nc.scalar.activation(osb[:, i % 2, :], pj[:], AF.Copy,
                                 scale=dinv[:])
            if i % 2 == 1:
                nc.sync.dma_start(out_d[:, i - 1:i + 1, :], osb[:])

        state = {"s_prev": s0_sb, "pending": None}

        def rec_chunk(i):
            """S-chain per-chunk work + pipelined tail of chunk i-1."""
            blk, sub = i // 4, i % 4
            ci = slice(i * C, (i + 1) * C)
            cs = slice(sub * C, (sub + 1) * C)
            s_prev = state["s_prev"]
            am = am_tiles.pop(i)
            # numerator psum tile; row 64 = denominator^T
            pon = psml.tile([HD + 1, 128], F32, tag="sml", name=f"pk{i}")
            po65 = pon[:]
            # S' += kp_tm^T @ [v|1]  (PSUM accumulation across chunks).
            # The final chunk's S-update/snapshot have no consumer -- skip
            # both (they sat on the critical tail's PE and DVE queues).
            if i < NCH - 1:
                nc.tensor.matmul(ps_s[:], kp_tm[:, ci], vv[:, i, :],
                                 start=(i == 0), stop=(i == NCH - 2),
                                 skip_group_check=True)
            # [num^T; den^T] [65, ti] = [S|z]^T qpc + [v|1]^T am.
            # Chunk 0: s_prev is all-zeros -- its matmul contributes exactly
            # nothing (including the den row: z0 = 0), so skip it.
            if i > 0:
                nc.tensor.matmul(po65, s_prev[:], qpT[:, ci],
                                 start=True, stop=False, skip_group_check=True)
            nc.tensor.matmul(po65, vv[:, i, :], am[:],
                             start=(i == 0), stop=True, skip_group_check=True)
            # S snapshot FIRST on DVE: it gates the whole next rec step
            # (num1 lhsT RAW + S-update WAR), while the num copy's only
            # consumers (pdc/outproj) are deferred a full chunk in the tail
            if i < NCH - 1:
                s_new = spool.tile([FEAT, HD + 1], BF16, name=f"s{i}")
                nc.vector.tensor_copy(s_new[:], ps_s[:])
            else:
                s_new = s_prev
            num = numpool.tile([HD + 1, 128], BF16, name=f"num{i}")
            nc.vector.tensor_copy(num[:], po65)
            # previous chunk's tail here: covers num-copy latency
            if state["pending"] is not None:
                emit_tail(*state["pending"])
            state["pending"] = (num, i)
            state["s_prev"] = s_new

        # ---- emission schedule (lag-2): the S-recursion of chunk i runs
        # two chunks behind its prep, so only TWO bare rec steps remain at
        # the end (the bare-rec tail is where the PE goes gappy and the HAM
        # clock-gate re-throttles to 1.2GHz -- measured 537ns N=512 MMs).
        # v/feat/prep matmuls are interleaved as filler between rec steps
        # so the PE stays dense while DVE/ACT service the S-chain. ----
        feat_mm(0, 0)
        v_mm(0)
        v_mm(1)
        feat_dve(0, 0)
        v_copy(0)
        v_copy(1)
        v_mm(2)
        v_mm(3)
        feat_mm(0, 1)
        feat_dve(0, 1)
        v_copy(2)
        v_copy(3)
        prep_chunk(0)
        prep_chunk(1)
        rec_chunk(0)
        prep_chunk(2)
        rec_chunk(1)
        prep_chunk(3)
        for blk in range(1, NBLK):
            feat_mm(blk, 0)
            rec_chunk(4 * blk - 2)
            v_mm(4 * blk + 0)
            rec_chunk(4 * blk - 1)
            v_copy(4 * blk + 0)
            v_mm(4 * blk + 1)
            feat_dve(blk, 0)
            feat_mm(blk, 1)
            v_copy(4 * blk + 1)
            feat_dve(blk, 1)
            prep_chunk(4 * blk + 0)
            prep_chunk(4 * blk + 1)
            rec_chunk(4 * blk + 0)
            v_mm(4 * blk + 2)
            rec_chunk(4 * blk + 1)
            v_copy(4 * blk + 2)
            v_mm(4 * blk + 3)
            prep_chunk(4 * blk + 2)
            prep_chunk(4 * blk + 3)
            v_copy(4 * blk + 3)
        rec_chunk(14)
        rec_chunk(15)
        emit_tail(*state["pending"])

    nc.compile()
    return nc


_cache = threading.Lock()
_nc = None


def _get_nc():
    global _nc
    with _cache:
        if _nc is None:
            _nc = build_nc()
    return _nc


def _in_maps(x, proj_matrix, W_qkv, W_out):
    # x^T pre-tiled: [512, 2048] -> [kt, 128, blk, 512] -> [128, blk, kt, 512]
    xT = np.ascontiguousarray(
        x[0].T.reshape(KT, 128, NBLK, 512).transpose(1, 2, 0, 3)
    ).astype(NP_BF16)
    mask = (np.arange(128)[:, None] <= np.arange(128)[None, :]).astype(np.float32)
    ident = np.eye(128, dtype=np.float32)

    def ktile(cols):
        # (512, m) -> (128, 4*m) k-tile layout, kt-major columns
        m = cols.shape[1]
        return cols.reshape(KT, 128, m).transpose(1, 0, 2).reshape(128, KT * m)

    maps = []
    for c in range(HEADS):
        pm = proj_matrix[c]                                 # (64, 128)
        wq = W_qkv[:, c * HD:(c + 1) * HD]                  # (512, 64)
        wk = W_qkv[:, DIM + c * HD: DIM + (c + 1) * HD]
        wv_ = W_qkv[:, 2 * DIM + c * HD: 2 * DIM + (c + 1) * HD]
        wcat = np.concatenate(
            [ktile(wq @ pm), ktile(wv_), ktile(wk @ pm), mask, ident],
            axis=1).astype(NP_BF16)
        maps.append({
            "xt": xT,
            "wcat": np.ascontiguousarray(wcat),
            "w_out_h": np.ascontiguousarray(
                W_out[c * HD:(c + 1) * HD, :]).astype(NP_BF16),
        })
    return maps


def kernel(x, proj_matrix, W_qkv, W_out, b_out, _trace=False):
    x = np.asarray(x, dtype=np.float32)
    proj_matrix = np.asarray(proj_matrix, dtype=np.float32)
    W_qkv = np.asarray(W_qkv, dtype=np.float32)
    W_out = np.asarray(W_out, dtype=np.float32)
    b_out = np.asarray(b_out, dtype=np.float32)

    nc = _get_nc()
    maps = _in_maps(x, proj_matrix, W_qkv, W_out)
    res = run_bass_kernel_spmd(nc, maps, core_ids=list(range(HEADS)), trace=_trace)
    out = np.zeros((N, DIM), dtype=np.float32)
    for r in res.results:
        part = np.asarray(r["out_part"], dtype=np.float32)   # [128, 16, 512]
        out += part.transpose(1, 0, 2).reshape(N, DIM)
    out += b_out
    if _trace:
        return out.reshape(1, N, DIM), res
    return out.reshape(1, N, DIM)

